# revision 1
# baseline (speedup 1.0000x reference)
"""Trainium2 Bass kernel for nn_DiffusionModuleV2 (dense transformer block).

Sharding: 8 cores = 2 batches x 4 query-quarters; fully token-parallel
(AdaLN, projections, FFN on the core's own 384 tokens) with one AllGather
per 4-core batch group for K/V.

Device layout: transposed activations [D-partitions (6x128 chunks), token-free].
All layout prep happens on host. Attention uses head-pair packing (dh=48 in
64-row slots, PE row/col tiling), additive positional bias added on DVE from a
host-gathered table (E = pos_weight[h, bins]), exp with fused accum_out
denominators on ScalarE, and DMA-transpose for the P@V operand. K/V are
computed per-quarter and all-gathered across each 4-core batch group.
"""

import sys

sys.path.insert(0, "/opt/trn_rl_repo")

import numpy as np
import ml_dtypes

BF = ml_dtypes.bfloat16
F32 = np.float32

B, N, D, H = 2, 1536, 768, 16
DH, DHP = 48, 64
FF = 4 * D
EPS = 1e-5
NCORES = 8
QPC = N // 4          # 384 queries per core
NCH = D // 128        # 6
FCH = FF // 128       # 24
HP = H // 2           # 8 head pairs
KCH = N // 512        # 3 key chunks of 512
QT = QPC // 128       # 3 query tiles of 128

_PROGRAM_CACHE = {}


def ts(start, size):
    return slice(start, start + size)


# ----------------------------------------------------------------------------
# host-side layout helpers
# ----------------------------------------------------------------------------

def _chunkT(x_t):  # (D, T) -> [128, NCH, T]
    d, t = x_t.shape
    return np.ascontiguousarray(x_t.reshape(d // 128, 128, t).transpose(1, 0, 2))


def _wtiles(w):  # (Din, Cout) -> [128, Din/128, Cout/128, 128]
    din, cout = w.shape
    return np.ascontiguousarray(
        w.reshape(din // 128, 128, cout // 128, 128).transpose(1, 0, 2, 3)
    )


def _colvec(v):  # (D,) per-out-col bias -> [128, NCH, 1]
    return np.ascontiguousarray(v.reshape(NCH, 128, 1).transpose(1, 0, 2)).astype(F32)


def _rowvec(v):  # (D,) -> [1, NCH, 128]  (K=1 matmul lhsT slices)
    return np.ascontiguousarray(v.reshape(1, NCH, 128)).astype(F32)


def _pad_qk(w):  # (D, H*48) -> (D, H*64), head h cols at 64h..64h+47
    out = np.zeros((D, H * DHP), w.dtype)
    for h in range(H):
        out[:, h * DHP : h * DHP + DH] = w[:, h * DH : (h + 1) * DH]
    return out


def _pad_wo(w):  # (H*48, D) -> (H*64, D), head h rows at 64h..64h+47
    out = np.zeros((H * DHP, D), w.dtype)
    for h in range(H):
        out[h * DHP : h * DHP + DH, :] = w[h * DH : (h + 1) * DH, :]
    return out


def prep_weights(inputs):
    w = {}
    f = lambda k: np.asarray(inputs[k], np.float64)

    def adaln(pfx, ln_w, ln_b, gw, gb, bw):
        gw_eff = (ln_w[:, None] * gw).astype(BF)
        bw_eff = (ln_w[:, None] * bw).astype(BF)
        w[pfx + "gw"] = _wtiles(gw_eff)
        w[pfx + "bw"] = _wtiles(bw_eff)
        w[pfx + "gb"] = _colvec(gb + ln_b @ gw)
        w[pfx + "bb"] = _colvec(ln_b @ bw)
        w[pfx + "csg"] = _rowvec(-gw_eff.astype(np.float64).sum(0))
        w[pfx + "csb"] = _rowvec(-bw_eff.astype(np.float64).sum(0))

    adaln("a1", f("a1_ln_w"), f("a1_ln_b"), f("a1_gw"), f("a1_gb"), f("a1_bw"))
    adaln("a2", f("a2_ln_w"), f("a2_ln_b"), f("a2_gw"), f("a2_gb"), f("a2_bw"))

    w["wq"] = _wtiles(_pad_qk((f("wq") / np.sqrt(DH)).astype(BF)))
    w["wk"] = _wtiles(_pad_qk(f("wk").astype(BF)))
    w["wv"] = _wtiles(f("wv").astype(BF))
    w["wg"] = _wtiles(f("wg").astype(BF))
    w["wo"] = _wtiles(_pad_wo(f("wo").astype(BF)))
    w["g1w"] = _wtiles(f("g1_w").astype(BF))
    w["g1b"] = _colvec(f("g1_b"))
    w["g2w"] = _wtiles(f("g2_w").astype(BF))
    w["g2b"] = _colvec(f("g2_b"))
    w["swg"] = _wtiles(f("sw_gate").astype(BF))
    w["swu"] = _wtiles(f("sw_up").astype(BF))
    w["swd"] = _wtiles(f("sw_down").astype(BF))
    return w


def host_prep(inputs):
    """Build the 8 per-core input maps (numpy, dtypes matching DRAM decls)."""
    wts = prep_weights(inputs)
    s = np.asarray(inputs["s"], F32)
    cond = np.asarray(inputs["s_cond"], F32)
    pw = np.asarray(inputs["pos_weight"], F32)  # (H, NBINS)
    bins = np.asarray(inputs["pos_bins"])

    in_maps = []
    for c in range(NCORES):
        b, qi = c // 4, c % 4
        qsl = slice(qi * QPC, (qi + 1) * QPC)
        m = dict(wts)
        m["sT"] = _chunkT(s[b].T[:, qsl]).astype(BF)
        m["cT"] = _chunkT(cond[b].T[:, qsl]).astype(BF)
        m["sqT"] = _chunkT(s[b].T[:, qsl]).astype(F32)
        bq = bins[b, qsl]                     # (QPC, N), keys global order
        m["E"] = np.ascontiguousarray(
            pw[:, bq].astype(BF).reshape(H, QT, 128, N))
        m["ident"] = np.eye(128, dtype=BF)
        in_maps.append(m)
    return in_maps


def assemble_output(results):
    out = np.empty((B, N, D), F32)
    for c in range(NCORES):
        b, qi = c // 4, c % 4
        t = np.asarray(results[c]["outT"])  # [128, NCH, QPC]
        out[b, qi * QPC : (qi + 1) * QPC, :] = (
            t.transpose(1, 0, 2).reshape(D, QPC).T)
    return out


# ----------------------------------------------------------------------------
# device program
# ----------------------------------------------------------------------------

def declare_io(nc, mybir):
    f32, bf16 = mybir.dt.float32, mybir.dt.bfloat16
    dram = {}

    def din(name, shape, dt):
        dram[name] = nc.dram_tensor(name, shape, dt, kind="ExternalInput")

    din("sT", [128, NCH, QPC], bf16)
    din("cT", [128, NCH, QPC], bf16)
    din("sqT", [128, NCH, QPC], f32)
    din("E", [H, QT, 128, N], bf16)
    din("ident", [128, 128], bf16)
    for pfx in ("a1", "a2"):
        din(pfx + "gw", [128, NCH, NCH, 128], bf16)
        din(pfx + "bw", [128, NCH, NCH, 128], bf16)
        din(pfx + "gb", [128, NCH, 1], f32)
        din(pfx + "bb", [128, NCH, 1], f32)
        din(pfx + "csg", [1, NCH, 128], f32)
        din(pfx + "csb", [1, NCH, 128], f32)
    din("wq", [128, NCH, HP, 128], bf16)
    din("wk", [128, NCH, HP, 128], bf16)
    din("wv", [128, NCH, NCH, 128], bf16)
    din("wg", [128, NCH, NCH, 128], bf16)
    din("wo", [128, HP, NCH, 128], bf16)
    din("g1w", [128, NCH, NCH, 128], bf16)
    din("g1b", [128, NCH, 1], f32)
    din("g2w", [128, NCH, NCH, 128], bf16)
    din("g2b", [128, NCH, 1], f32)
    din("swg", [128, NCH, FCH, 128], bf16)
    din("swu", [128, NCH, FCH, 128], bf16)
    din("swd", [128, FCH, NCH, 128], bf16)
    dram["outT"] = nc.dram_tensor("outT", [128, NCH, QPC], f32,
                                  kind="ExternalOutput")
    return dram


def build_program():
    import concourse.mybir as mybir
    import concourse.tile as tile
    from concourse import bacc

    nc = bacc.Bacc("TRN2", target_bir_lowering=False, debug=False,
                   num_devices=NCORES)
    dram = declare_io(nc, mybir)
    with tile.TileContext(nc) as tc:
        _emit(nc, tc, dram, mybir)
    nc.compile()
    return nc


def _emit(nc, tc, dram, mybir):
    import contextlib

    f32, bf16 = mybir.dt.float32, mybir.dt.bfloat16
    AF = mybir.ActivationFunctionType
    OP = mybir.AluOpType

    ctx = contextlib.ExitStack()
    with ctx:
        const = ctx.enter_context(tc.tile_pool(name="const", bufs=1))
        outer = ctx.enter_context(tc.tile_pool(name="outer", bufs=1))

        # ---- constants / small residents (allocate everything up front) ----
        ones_bf = const.tile([128, 1], bf16, tag="ones_bf")
        nc.vector.memset(ones_bf[:], 1.0)
        ones_f1 = const.tile([1, 128], f32, tag="ones_f1")
        nc.vector.memset(ones_f1[:], 1.0)

        cvec = {}
        for name in ("a1gb", "a1bb", "a1csg", "a1csb", "a2gb", "a2bb",
                     "a2csg", "a2csb", "g1b", "g2b"):
            t = const.tile(list(dram[name].shape), dram[name].dtype,
                           name="c_" + name, tag=name)
            nc.sync.dma_start(out=t[:], in_=dram[name][:])
            cvec[name] = t

        ident = const.tile([128, 128], bf16, tag="ident")
        nc.sync.dma_start(out=ident[:], in_=dram["ident"][:])
        eps1 = const.tile([1, 1], f32, tag="eps1")
        nc.vector.memset(eps1[:], EPS)
        mrow_c = const.tile([1, QPC], f32, tag="mrow_c")

        # ---- persistent activations ----
        cT = outer.tile([128, NCH, QPC], bf16, tag="cT")
        nc.sync.dma_start(out=cT[:], in_=dram["cT"][:])
        sqT = outer.tile([128, NCH, QPC], f32, tag="sqT")
        nc.sync.dma_start(out=sqT[:], in_=dram["sqT"][:])
        s_new = outer.tile([128, NCH, QPC], f32, tag="s_new")
        Rs_c = outer.tile([128, QPC], f32, tag="Rs_c")

        # ------------------------------------------------------------------
        def ln_stats(x_bf, T, m_row, r_row, tag):
            """LN stats over the partition (D) axis -> m_row, r_row [1, T]."""
            with tc.tile_pool(name="st_" + tag, bufs=3) as wp, \
                 tc.tile_pool(name="stp_" + tag, bufs=2, space="PSUM") as pp:
                for t in range((T + 511) // 512):
                    w = min(512, T - t * 512)
                    tsl = ts(t * 512, w)
                    ps1 = pp.tile([1, 512], f32, tag="ps1")
                    ps2 = pp.tile([1, 512], f32, tag="ps2")
                    for ch in range(NCH):
                        sq = wp.tile([128, 512], bf16, tag="sq")
                        nc.vector.tensor_mul(sq[:, :w], x_bf[:, ch, tsl],
                                             x_bf[:, ch, tsl])
                        nc.tensor.matmul(ps1[:, :w], ones_bf[:], x_bf[:, ch, tsl],
                                         start=(ch == 0), stop=(ch == NCH - 1))
                        nc.tensor.matmul(ps2[:, :w], ones_bf[:], sq[:, :w],
                                         start=(ch == 0), stop=(ch == NCH - 1))
                    mm = m_row[:, tsl]
                    nc.vector.tensor_scalar_mul(mm, ps1[:, :w], 1.0 / D)
                    msq = wp.tile([1, 512], f32, tag="msq", bufs=1)
                    nc.vector.tensor_mul(msq[:, :w], mm, mm)
                    v = wp.tile([1, 512], f32, tag="v", bufs=1)
                    nc.vector.scalar_tensor_tensor(
                        v[:, :w], ps2[:, :w], 1.0 / D, msq[:, :w],
                        op0=OP.mult, op1=OP.subtract)
                    lnv = wp.tile([1, 512], f32, tag="lnv", bufs=1)
                    nc.scalar.activation(lnv[:, :w], v[:, :w], AF.Ln,
                                         bias=eps1[:])
                    nc.scalar.activation(r_row[:, tsl], lnv[:, :w], AF.Exp,
                                         scale=-0.5)

        def bcast_rows(pairs, T, pp):
            """Replicate [1, T] rows to [128, T] SBUF via K=1 PE matmuls."""
            for row, dst in pairs:
                for t in range((T + 511) // 512):
                    w = min(512, T - t * 512)
                    tsl = ts(t * 512, w)
                    ps = pp.tile([128, 512], f32, tag="bc")
                    nc.tensor.matmul(ps[:, :w], ones_f1[:], row[:, tsl],
                                     start=True, stop=True)
                    nc.scalar.copy(dst[:, tsl], ps[:, :w])

        def ln_apply(x_bf, T, m_row, R_sb, xn, pp, wp):
            """xn = (x - M) * R with M broadcast via PE, R from SBUF."""
            for t in range((T + 511) // 512):
                w = min(512, T - t * 512)
                tsl = ts(t * 512, w)
                psM = pp.tile([128, 512], f32, tag="psM")
                nc.tensor.matmul(psM[:, :w], ones_f1[:], m_row[:, tsl],
                                 start=True, stop=True)
                for ch in range(NCH):
                    d = wp.tile([128, 512], f32, tag="d")
                    nc.vector.tensor_sub(d[:, :w], x_bf[:, ch, tsl], psM[:, :w])
                    nc.vector.tensor_mul(xn[:, ch, tsl], d[:, :w], R_sb[:, tsl])

        def adaln_gb(pfx, cond_tile, xn, R_c, m_c, T, sn_out,
                     gw_all=None, bw_all=None):
            """sn = sigmoid(psG*R_c + gb) * xn + (psB*R_c + bb).

            psG/psB = W^T @ cond_raw + (-colsum) (x) m_c  (mean fold).
            cond_tile/R_c/m_c are full tiles; only tokens [0, T) are used.
            """
            gb, bb = cvec[pfx + "gb"], cvec[pfx + "bb"]
            csg, csb = cvec[pfx + "csg"], cvec[pfx + "csb"]
            with tc.tile_pool(name=pfx + "w", bufs=2) as wp, \
                 tc.tile_pool(name=pfx + "t", bufs=3) as tp, \
                 tc.tile_pool(name=pfx + "p", bufs=2, space="PSUM") as pp:
                for co in range(NCH):
                    if gw_all is not None:
                        gwc, bwc = gw_all[:, :, co, :], bw_all[:, :, co, :]
                    else:
                        gwc = wp.tile([128, NCH, 128], bf16, tag="gwc")
                        nc.sync.dma_start(out=gwc[:],
                                          in_=dram[pfx + "gw"][:, :, co, :])
                        bwc = wp.tile([128, NCH, 128], bf16, tag="bwc")
                        nc.sync.dma_start(out=bwc[:],
                                          in_=dram[pfx + "bw"][:, :, co, :])
                    for t in range((T + 511) // 512):
                        w = min(512, T - t * 512)
                        tsl = ts(t * 512, w)
                        psg = pp.tile([128, 512], f32, tag="psg")
                        psb = pp.tile([128, 512], f32, tag="psb")
                        for ci in range(NCH):
                            nc.tensor.matmul(psg[:, :w], gwc[:, ci, :],
                                             cond_tile[:, ci, tsl],
                                             start=(ci == 0), stop=False)
                            nc.tensor.matmul(psb[:, :w], bwc[:, ci, :],
                                             cond_tile[:, ci, tsl],
                                             start=(ci == 0), stop=False)
                        nc.tensor.matmul(psg[:, :w], csg[:, co, :],
                                         m_c[:, tsl], start=False, stop=True)
                        nc.tensor.matmul(psb[:, :w], csb[:, co, :],
                                         m_c[:, tsl], start=False, stop=True)
                        gr = tp.tile([128, 512], f32, tag="gr")
                        nc.vector.tensor_mul(gr[:, :w], psg[:, :w], R_c[:, tsl])
                        sig = tp.tile([128, 512], bf16, tag="sig")
                        nc.scalar.activation(sig[:, :w], gr[:, :w], AF.Sigmoid,
                                             bias=gb[:, co, :])
                        br = tp.tile([128, 512], f32, tag="br")
                        nc.vector.tensor_mul(br[:, :w], psb[:, :w], R_c[:, tsl])
                        t1 = tp.tile([128, 512], bf16, tag="t1")
                        nc.vector.tensor_mul(t1[:, :w], sig[:, :w], xn[:, co, tsl])
                        nc.vector.scalar_tensor_tensor(
                            sn_out[:, co, tsl], br[:, :w], bb[:, co, :],
                            t1[:, :w], op0=OP.add, op1=OP.add)

        # ==================================================================
        # Phase A: AdaLN1 over the full batch -> snT
        # ==================================================================
        attstack = contextlib.ExitStack()
        pSn = attstack.enter_context(tc.tile_pool(name="pSn", bufs=1))
        snT = pSn.tile([128, NCH, QPC], bf16, tag="snT")
        with tc.tile_pool(name="pA", bufs=1) as pA:
            sT = pA.tile([128, NCH, QPC], bf16, tag="sT")
            nc.sync.dma_start(out=sT[:], in_=dram["sT"][:])
            mrow_s = pA.tile([1, QPC], f32, tag="mrow_s")
            rrow_s = pA.tile([1, QPC], f32, tag="rrow_s")
            rrow_c = pA.tile([1, QPC], f32, tag="rrow_c")
            a1gw_all = pA.tile([128, NCH, NCH, 128], bf16, tag="a1gw_all")
            nc.sync.dma_start(out=a1gw_all[:], in_=dram["a1gw"][:])
            a1bw_all = pA.tile([128, NCH, NCH, 128], bf16, tag="a1bw_all")
            nc.sync.dma_start(out=a1bw_all[:], in_=dram["a1bw"][:])
            ln_stats(sT, QPC, mrow_s, rrow_s, "s")
            ln_stats(cT, QPC, mrow_c, rrow_c, "c")

            xn = pA.tile([128, NCH, QPC], bf16, tag="xn")
            Rs_s = pA.tile([128, QPC], f32, tag="Rs_s")
            with tc.tile_pool(name="bcA", bufs=2, space="PSUM") as ppb, \
                 tc.tile_pool(name="bcAw", bufs=3) as bw:
                bcast_rows([(rrow_c, Rs_c), (rrow_s, Rs_s)], QPC, ppb)
                ln_apply(sT, QPC, mrow_s, Rs_s, xn, ppb, bw)
            adaln_gb("a1", cT, xn, Rs_c, mrow_c, QPC, snT,
                     gw_all=a1gw_all, bw_all=a1bw_all)

        # ==================================================================
        # Phase B: Q/K/V/G projections
        # ==================================================================
        pAtt = attstack.enter_context(tc.tile_pool(name="pAtt", bufs=1))
        Kt = pAtt.tile([128, HP, N], bf16, tag="Kt")
        Qt = pAtt.tile([128, HP, QPC], bf16, tag="Qt")
        Vt = pAtt.tile([128, 12, D], bf16, tag="Vt")
        sig_g = pAtt.tile([128, NCH, QPC], bf16, tag="sig_g")
        att_nT = pAtt.tile([128, HP, QPC], bf16, tag="att_nT")
        nc.vector.memset(att_nT[:], 0.0)
        with tc.tile_pool(name="pB", bufs=2) as pB, \
             tc.tile_pool(name="pBw", bufs=1) as pBw, \
             tc.tile_pool(name="pBp", bufs=2, space="PSUM") as pBp:
            KB = HP * QPC          # 3072 bf16 per partition
            VB = QT * D            # 2304
            dp = attstack.enter_context(
                tc.tile_pool(name="ccd", bufs=1, space="DRAM"))
            kc_in = dp.tile([128, KB], bf16, name="kc_in")
            kc_out = dp.tile([4, 128, KB], bf16, name="kc_out")
            vc_in = dp.tile([128, VB], bf16, name="vc_in")
            vc_out = dp.tile([4, 128, VB], bf16, name="vc_out")
            Ktl = pB.tile([128, HP, QPC], bf16, tag="Ktl", bufs=1)
            for name, dst, T in (("wk", Ktl, QPC),):
                for hp in range(HP):
                    wc = pB.tile([128, NCH, 128], bf16, tag="wc")
                    nc.sync.dma_start(out=wc[:], in_=dram[name][:, :, hp, :])
                    for t in range((T + 511) // 512):
                        w = min(512, T - t * 512)
                        tsl = ts(t * 512, w)
                        ps = pBp.tile([128, 512], f32, tag="ps")
                        for ci in range(NCH):
                            nc.tensor.matmul(ps[:, :w], wc[:, ci, :],
                                             snT[:, ci, tsl],
                                             start=(ci == 0), stop=(ci == NCH - 1))
                        nc.vector.tensor_copy(dst[:, hp, tsl], ps[:, :w])
            nc.sync.dma_start(out=kc_in[:],
                              in_=Ktl[:].rearrange("p a b -> p (a b)"))
            nc.gpsimd.collective_compute(
                "AllGather", mybir.AluOpType.bypass,
                replica_groups=[[0, 1, 2, 3], [4, 5, 6, 7]],
                ins=[kc_in[:]], outs=[kc_out[:]])
            wv_all = pBw.tile([128, NCH, NCH, 128], bf16, tag="wv_all")
            nc.sync.dma_start(out=wv_all[:], in_=dram["wv"][:])
            Vtl = pB.tile([128, QT, D], bf16, tag="Vtl", bufs=1)
            for tt in range(QT):
                for cg in range(2):
                    psv = pBp.tile([128, 384], f32, tag="psv")
                    for ci in range(NCH):
                        nc.tensor.matmul(psv[:], snT[:, ci, ts(tt * 128, 128)],
                                         wv_all[:, ci, ts(cg * 3, 3)],
                                         start=(ci == 0), stop=(ci == NCH - 1))
                    nc.vector.tensor_copy(Vtl[:, tt, ts(cg * 384, 384)], psv[:])
            nc.sync.dma_start(out=vc_in[:],
                              in_=Vtl[:].rearrange("p a b -> p (a b)"))
            nc.gpsimd.collective_compute(
                "AllGather", mybir.AluOpType.bypass,
                replica_groups=[[0, 1, 2, 3], [4, 5, 6, 7]],
                ins=[vc_in[:]], outs=[vc_out[:]])
            # overlap the collectives with Q/G projections
            for name, dst, T in (("wq", Qt, QPC),):
                for hp in range(HP):
                    wc = pB.tile([128, NCH, 128], bf16, tag="wc")
                    nc.sync.dma_start(out=wc[:], in_=dram[name][:, :, hp, :])
                    for t in range((T + 511) // 512):
                        w = min(512, T - t * 512)
                        tsl = ts(t * 512, w)
                        ps = pBp.tile([128, 512], f32, tag="ps")
                        for ci in range(NCH):
                            nc.tensor.matmul(ps[:, :w], wc[:, ci, :],
                                             snT[:, ci, tsl],
                                             start=(ci == 0), stop=(ci == NCH - 1))
                        nc.vector.tensor_copy(dst[:, hp, tsl], ps[:, :w])
            for r in range(4):
                nc.sync.dma_start(
                    out=Kt[:, :, ts(r * QPC, QPC)],
                    in_=kc_out[r].rearrange("p (a b) -> p a b", a=HP))
            wg_all = pBw.tile([128, NCH, NCH, 128], bf16, tag="wg_all")
            nc.sync.dma_start(out=wg_all[:], in_=dram["wg"][:])
            for co in range(NCH):
                psgf = pBp.tile([128, 384], f32, tag="psgf")
                for ci in range(NCH):
                    nc.tensor.matmul(psgf[:], wg_all[:, ci, co, :],
                                     snT[:, ci, 0:QPC],
                                     start=(ci == 0), stop=(ci == NCH - 1))
                nc.scalar.activation(sig_g[:, co, :], psgf[:], AF.Sigmoid)
            for r in range(4):
                nc.sync.dma_start(
                    out=Vt[:, ts(r * QT, QT), :],
                    in_=vc_out[r].rearrange("p (a b) -> p a b", a=QT))

        # ==================================================================
        # Phase C: attention per head pair -> att_nT
        # ==================================================================
        with tc.tile_pool(name="pC", bufs=2) as pC, \
             tc.tile_pool(name="pCw", bufs=1) as pCw, \
             tc.tile_pool(name="pCp", bufs=2, space="PSUM") as pCp:
            def emit_pv(hp, WT):
                attp = pCp.tile([128, QPC], f32, tag="attp", name="attp")
                for kt in range(12):
                    nc.tensor.matmul(attp[0:DH, :],
                                     Vt[:, kt, ts(2 * hp * DH, DH)],
                                     WT["A"][:, kt, :],
                                     start=(kt == 0), stop=(kt == 11),
                                     tile_position=(0, 0),
                                     skip_group_check=True)
                    nc.tensor.matmul(attp[64 : 64 + DH, :],
                                     Vt[:, kt, ts((2 * hp + 1) * DH, DH)],
                                     WT["B"][:, kt, :],
                                     start=(kt == 0), stop=(kt == 11),
                                     tile_position=(0, 64),
                                     skip_group_check=True)
                for h, plo in ((2 * hp, 0), (2 * hp + 1, 64)):
                    off = (h % 2) * 64
                    nc.scalar.copy(att_nT[off : off + DH, h // 2, :],
                                   attp[plo : plo + DH, :])

            prev = None
            for hp in range(HP):
                WT = {}
                for h, plo, side in ((2 * hp, 0, "A"), (2 * hp + 1, 64, "B")):
                    WT[side] = pCw.tile([128, 12, QPC], bf16,
                                        name="WT" + side, tag="WT" + side,
                                        bufs=2)
                    for qt in range(QT):
                        qsl = ts(qt * 128, 128)
                        E_t = pC.tile([128, N], bf16, tag="Et", bufs=3)
                        nc.sync.dma_start(out=E_t[:], in_=dram["E"][h, qt])
                        W = pC.tile([128, N], bf16, tag="W")
                        den = pC.tile([128, 1], f32, tag="den")
                        pss = pCp.tile([128, KCH, 512], f32, tag="pss",
                                       bufs=2)
                        for kc in range(KCH):
                            ksl = ts(kc * 512, 512)
                            nc.tensor.matmul(pss[:, kc, :],
                                             Qt[plo : plo + DH, hp, qsl],
                                             Kt[plo : plo + DH, hp, ksl],
                                             start=True, stop=True)
                        W0 = pC.tile([128, N], bf16, tag="W0")
                        nc.vector.tensor_add(
                            W0[:].rearrange("p (a b) -> p a b", a=KCH),
                            pss[:],
                            E_t[:].rearrange("p (a b) -> p a b", a=KCH))
                        nc.scalar.activation(W[:], W0[:], AF.Exp,
                                             accum_out=den[:])
                        rd = pC.tile([128, 1], f32, tag="rd")
                        nc.vector.reciprocal(rd[:], den[:])
                        Wn = pC.tile([128, N], bf16, tag="Wn")
                        nc.vector.tensor_scalar_mul(Wn[:], W[:], rd[:])
                        nc.sync.dma_start(out=WT[side][:, :, qsl], in_=Wn[:],
                                          transpose=True)
                if prev is not None:
                    emit_pv(hp - 1, prev)
                prev = WT
            emit_pv(HP - 1, prev)

        # ==================================================================
        # Phase D: wo + gates + residual -> s_new
        # ==================================================================
        with tc.tile_pool(name="pD", bufs=2) as pD, \
             tc.tile_pool(name="pDw", bufs=1) as pDw, \
             tc.tile_pool(name="pDp", bufs=2, space="PSUM") as pDp:
            wo_all = pDw.tile([128, HP, NCH, 128], bf16, tag="wo_all")
            nc.sync.dma_start(out=wo_all[:], in_=dram["wo"][:])
            g1_all = pDw.tile([128, NCH, NCH, 128], bf16, tag="g1_all")
            nc.sync.dma_start(out=g1_all[:], in_=dram["g1w"][:])
            for co in range(NCH):
                pso = pDp.tile([128, QPC], f32, tag="pso")
                ps1 = pDp.tile([128, QPC], f32, tag="ps1")
                for ci in range(HP):
                    nc.tensor.matmul(pso[:], wo_all[:, ci, co, :],
                                     att_nT[:, ci, :],
                                     start=(ci == 0), stop=(ci == HP - 1))
                for ci in range(NCH):
                    nc.tensor.matmul(ps1[:], g1_all[:, ci, co, :],
                                     cT[:, ci, :],
                                     start=(ci == 0), stop=(ci == NCH - 1))
                upd = pD.tile([128, QPC], bf16, tag="upd")
                nc.vector.tensor_mul(upd[:], sig_g[:, co, :], pso[:])
                sig1 = pD.tile([128, QPC], bf16, tag="sig1")
                nc.scalar.activation(sig1[:], ps1[:], AF.Sigmoid,
                                     bias=cvec["g1b"][:, co, :])
                t2 = pD.tile([128, QPC], bf16, tag="t2")
                nc.vector.tensor_mul(t2[:], sig1[:], upd[:])
                nc.vector.tensor_add(s_new[:, co, :], sqT[:, co, :], t2[:])

        attstack.close()  # free snT/Kt/Qt/Vt/sig_g/att_nT

        # ==================================================================
        # Phase E: AdaLN2 (quarter) -> sn2
        # ==================================================================
        pEF = ctx.enter_context(tc.tile_pool(name="pEF", bufs=1))
        sn2 = pEF.tile([128, NCH, QPC], bf16, tag="sn2")
        with tc.tile_pool(name="pE", bufs=1) as pE:
            xb2 = pE.tile([128, NCH, QPC], bf16, tag="xb2")
            for ch in range(NCH):
                nc.scalar.copy(xb2[:, ch, :], s_new[:, ch, :])
            mrow2 = pE.tile([1, QPC], f32, tag="mrow2")
            rrow2 = pE.tile([1, QPC], f32, tag="rrow2")
            ln_stats(xb2, QPC, mrow2, rrow2, "s2")
            xn2 = pE.tile([128, NCH, QPC], bf16, tag="xn2")
            Rs2 = pE.tile([128, QPC], f32, tag="Rs2")
            with tc.tile_pool(name="bcE", bufs=2, space="PSUM") as ppb, \
                 tc.tile_pool(name="bcEw", bufs=3) as bw2:
                bcast_rows([(rrow2, Rs2)], QPC, ppb)
                ln_apply(xb2, QPC, mrow2, Rs2, xn2, ppb, bw2)
            a2gw_all = pE.tile([128, NCH, NCH, 128], bf16, tag="a2gw_all")
            nc.sync.dma_start(out=a2gw_all[:], in_=dram["a2gw"][:])
            a2bw_all = pE.tile([128, NCH, NCH, 128], bf16, tag="a2bw_all")
            nc.sync.dma_start(out=a2bw_all[:], in_=dram["a2bw"][:])
            adaln_gb("a2", cT, xn2, Rs_c, mrow_c, QPC, sn2,
                     gw_all=a2gw_all, bw_all=a2bw_all)

        # ==================================================================
        # Phase F: SwiGLU + g2 gate + residual -> outT
        # ==================================================================
        with tc.tile_pool(name="pF", bufs=3) as pF, \
             tc.tile_pool(name="pFh", bufs=1) as pFh, \
             tc.tile_pool(name="pFp", bufs=2, space="PSUM") as pFp:
            hT = pFh.tile([128, FCH, QPC], bf16, tag="hT")
            for co in range(FCH):
                gwc = pF.tile([128, NCH, 128], bf16, tag="gwc")
                nc.sync.dma_start(out=gwc[:], in_=dram["swg"][:, :, co, :])
                uwc = pF.tile([128, NCH, 128], bf16, tag="uwc")
                nc.sync.dma_start(out=uwc[:], in_=dram["swu"][:, :, co, :])
                psG = pFp.tile([128, QPC], f32, tag="psG")
                psU = pFp.tile([128, QPC], f32, tag="psU")
                for ci in range(NCH):
                    nc.tensor.matmul(psG[:], gwc[:, ci, :], sn2[:, ci, :],
                                     start=(ci == 0), stop=(ci == NCH - 1))
                    nc.tensor.matmul(psU[:], uwc[:, ci, :], sn2[:, ci, :],
                                     start=(ci == 0), stop=(ci == NCH - 1))
                sg = pF.tile([128, QPC], bf16, tag="sg")
                nc.scalar.activation(sg[:], psG[:], AF.Sigmoid)
                tg = pF.tile([128, QPC], bf16, tag="tg")
                nc.vector.tensor_mul(tg[:], sg[:], psG[:])
                nc.vector.tensor_mul(hT[:, co, :], tg[:], psU[:])
            outT = pFh.tile([128, NCH, QPC], f32, tag="outT")
            g2_all = pFh.tile([128, NCH, NCH, 128], bf16, tag="g2_all")
            nc.sync.dma_start(out=g2_all[:], in_=dram["g2w"][:])
            for co in range(NCH):
                dwc = pF.tile([128, FCH, 128], bf16, tag="dwc")
                nc.sync.dma_start(out=dwc[:], in_=dram["swd"][:, :, co, :])
                psD = pFp.tile([128, QPC], f32, tag="psD")
                for ki in range(FCH):
                    nc.tensor.matmul(psD[:], dwc[:, ki, :], hT[:, ki, :],
                                     start=(ki == 0), stop=(ki == FCH - 1))
                ps2 = pFp.tile([128, QPC], f32, tag="ps2")
                for ci in range(NCH):
                    nc.tensor.matmul(ps2[:], g2_all[:, ci, co, :],
                                     cT[:, ci, :],
                                     start=(ci == 0), stop=(ci == NCH - 1))
                sig2 = pF.tile([128, QPC], bf16, tag="sig2")
                nc.scalar.activation(sig2[:], ps2[:], AF.Sigmoid,
                                     bias=cvec["g2b"][:, co, :])
                t3 = pF.tile([128, QPC], bf16, tag="t3")
                nc.vector.tensor_mul(t3[:], sig2[:], psD[:])
                nc.vector.tensor_add(outT[:, co, :], s_new[:, co, :], t3[:])
            nc.sync.dma_start(out=dram["outT"][:], in_=outT[:])


# ----------------------------------------------------------------------------
# public entry point
# ----------------------------------------------------------------------------

def get_program():
    if "nc" not in _PROGRAM_CACHE:
        _PROGRAM_CACHE["nc"] = build_program()
    return _PROGRAM_CACHE["nc"]


def kernel(**inputs):
    from concourse.bass_utils import run_bass_kernel_spmd

    nc = get_program()
    in_maps = host_prep(inputs)
    res = run_bass_kernel_spmd(nc, in_maps, list(range(NCORES)))
    return assemble_output(res.results)


if __name__ == "__main__":
    import reference

    inputs = {k: np.asarray(v) for k, v in reference.setup_inputs().items()}
    out = kernel(**inputs)
    print("kernel output", out.shape, out.dtype)

